# revision 41
# baseline (speedup 1.0000x reference)
"""Trainium2 Bass kernel for nn_BiholoModelFuncGENERALforHYMinv3.

Computation (per sample):
  x[18] -> 9 complex coords in 3 projective factors of 3
  bihom feature chain -> sec[729] (divided by kappa product)
  two towers: u1=(sec@W1+b1)^2 -> u2=(.@W2+b2)^2 -> u3=(.@W3+b3)^2
  out = Wfa*log(u3a) - Wfb*log(u3b), clipped to +-1e6

Distribution: pure data parallel over batch, 8 NeuronCores, 4096 samples
per core. Weights replicated.

On-chip layout: features/hidden units on the partition axis, batch on the
free axis, so weights slice natively as matmul lhsT (m-major [K,M] chunks
of the [in,out]-shaped DRAM arrays; W1's rows are permuted host-side to
the kernel's featsT row order g = j*81 + i and zero-padded to 768). The
bihom chain runs on DVE with batch on partitions via broadcast-AP outer
products up to the level-2 vector L2[81] and kappa-folded factor-2 vector
f2[9]; one broadcast mul forms sec[128, 768] in fp8 (scale 128 folded
into 1/kappa), six PE transposes per 128-sample subtile flip it to
feature-on-partition, and one strided DVE copy per subtile assembles
featsT. The final W3 contraction runs on the (otherwise idle) GpSimd
engine: v[p,b] += W3[p,m] * q2[p,m,b] per m-chunk (per-partition-scalar
STT), with a single ones-vector PE matmul per tower/tile for the
partition sum — this keeps the PE stream to just the roofline L1/L2 fp8
DoubleRow matmuls plus transposes. The ones-matmuls and epilogues are
deferred past the next block's matmuls so the in-order PE queue never
head-of-line blocks on the ACT->GpSimd chain. The big L1/L2 matmuls run
as fp8e4 DoubleRow (2 contraction rows/cycle, 2x the fp32r rate);
scaling keeps operands in e4m3 range and is folded into existing
ACT/DVE ops. End-to-end output error ~7e-3 (tolerance 2e-2).
"""
import numpy as np

N_CORES = 8
B_FULL = 32768
B_CORE = B_FULL // N_CORES
N_TILE = 512          # moving-dim per tower pass (<= 512 for fp32 PSUM bank)
H = 1024              # hidden width
NSEC = 729
SEC_CHUNKS = [128, 128, 128, 128, 128, 89]   # 729 = 5*128 + 89
MM_DTYPE = "f32r"     # "f32r" | "f32" — PE matmul operand mode (L3/select path)
# L1/L2 run as fp8e4 DoubleRow matmuls (2 contraction rows/cycle). Scales keep
# operands inside e4m3's [2^-9, 240] band; all are folded into existing ops:
#   featsT stored as 128*sec (|sec|<=1 by Cauchy-Schwarz, so max 128 < 240)
#   W1, W2 stored as 32*W (w ~ N(0,0.05) -> rms 1.6)
#   q1 stored as 512*q1 = (sqrt(512)*(h1+b1))^2  (max(512*q1) ~ 53 measured)
# ACT de-scales: q1 scale sqrt(512)/(128*32), q2 scale 1/(512*32).
FP8_SF = 128.0
FP8_SW = 32.0
FP8_SQ1 = 512.0


def _brd(t_ap, free_dims, import_bass):
    """AP with t_ap's partition dim plus custom free [step,count] dims."""
    bass = import_bass
    return bass.AP(tensor=t_ap.tensor, offset=t_ap.offset,
                   ap=[list(t_ap.ap[0])] + [list(d) for d in free_dims])


def build_nc(b_core=B_CORE, n_tile=N_TILE, mm_dtype=MM_DTYPE, finalize=True):
    import concourse.bass as bass
    import concourse.tile as tile
    from concourse import mybir, bacc

    F32 = mybir.dt.float32
    F32R = mybir.dt.float32r
    FP8 = mybir.dt.float8e4
    AF = mybir.ActivationFunctionType
    ALU = mybir.AluOpType
    DR = mybir.MatmulPerfMode.DoubleRow

    MMDT = F32R if mm_dtype == "f32r" else F32

    assert b_core % n_tile == 0 and n_tile % 128 == 0
    n_macro = b_core // n_tile
    n_sub = n_tile // 128

    nc = bacc.Bacc()
    n_sub_total = b_core // 128
    x_d = nc.declare_dram_parameter("x", [128, n_sub_total * 18], F32, isOutput=False)
    wd = {}
    for t in ("a", "b"):
        # all weights pre-tiled host-side to the exact SBUF layouts so every
        # DMA is a simple contiguous pattern (fast SP issue, full HBM bursts).
        # b1 is folded into W1's zero-pad row 729 (sec col 729 is a constant
        # 128), so L1's ACT square needs no bias and can run m-chunk pairs.
        wd["W1" + t] = nc.declare_dram_parameter("W1" + t, [128, 8, 6, 128], FP8, isOutput=False)
        wd["W2" + t] = nc.declare_dram_parameter("W2" + t, [128, 8, 8, 128], FP8, isOutput=False)
        wd["b2" + t] = nc.declare_dram_parameter("b2" + t, [128, 8], F32, isOutput=False)
        wd["W3" + t] = nc.declare_dram_parameter("W3" + t, [128, 8], F32, isOutput=False)
        wd["b3" + t] = nc.declare_dram_parameter("b3" + t, [1], F32, isOutput=False)
        wd["Wf" + t] = nc.declare_dram_parameter("Wf" + t, [1, 1], F32, isOutput=False)
    out_d = nc.declare_dram_parameter("out", [b_core], F32, isOutput=True)

    with tile.TileContext(nc) as tc:
        import contextlib
        with contextlib.ExitStack() as ctx:
            consts = ctx.enter_context(tc.tile_pool(name="consts", bufs=1))
            xp = ctx.enter_context(tc.tile_pool(name="xp", bufs=4))
            ft = ctx.enter_context(tc.tile_pool(name="ft", bufs=1))
            ftp = ctx.enter_context(tc.tile_pool(name="ftp", bufs=2))
            secp = ctx.enter_context(tc.tile_pool(name="secp", bufs=4))
            ftsp = ctx.enter_context(tc.tile_pool(name="ftsp", bufs=4))
            qp = ctx.enter_context(tc.tile_pool(name="qp", bufs=2))
            q2p = ctx.enter_context(tc.tile_pool(name="q2p", bufs=2))
            vp = ctx.enter_context(tc.tile_pool(name="vp", bufs=2))
            ep = ctx.enter_context(tc.tile_pool(name="ep", bufs=1))
            psL = ctx.enter_context(tc.tile_pool(name="psL", bufs=3, space="PSUM"))
            psU = ctx.enter_context(tc.tile_pool(name="psU", bufs=2, space="PSUM"))

            # ---- constants / weights (resident) ----
            x_sb = consts.tile([128, n_sub_total * 18], F32, tag="x_sb", name="x_sb")

            W1 = {}; W2 = {}; W3 = {}; B1 = {}; B2 = {}; B3 = {}; WF = {}
            scal4 = consts.tile([1, 4], F32, tag="scal4", name="scal4")
            B3["a"] = scal4[0:1, 0:1]; B3["b"] = scal4[0:1, 1:2]
            WF["a"] = scal4[0:1, 2:3]; WF["b"] = scal4[0:1, 3:4]
            # ones column for the u3 partition-sum matmul (bf16: 1-pass PE)
            BF16 = mybir.dt.bfloat16
            ones = consts.tile([128, 1], BF16, tag="ones", name="ones")
            nc.vector.memset(ones[:], 1.0)
            for t in ("a", "b"):
                # m-major weight tiles: each (m, k-pair) piece is one small
                # DMA the consumer matmul can wait on individually
                W1[t] = consts.tile([128, 8, 6, 128], FP8, tag="W1" + t, name="W1" + t)
                W2[t] = consts.tile([128, 8, 8, 128], FP8, tag="W2" + t, name="W2" + t)
                W3[t] = consts.tile([128, 8], F32, tag="W3" + t, name="W3" + t)
                B2[t] = consts.tile([128, 8], F32, tag="b2" + t, name="b2" + t)

            # DMA issue order tracks first use (macro-0 x and W1 first) so
            # the 16 queues fill the startup-critical pieces before the
            # bulk. All weight reads are contiguous (host pre-tiled). The
            # bulk W2 block is emitted AFTER macro-0's feats (see below) so
            # its transfers don't sit ahead of the startup-critical sec
            # DMA-transposes in the hardware DMA queues.
            nc.sync.dma_start(out=x_sb[:, 0:4 * 18], in_=x_d[:, 0:4 * 18])
            for t in ("a", "b"):
                nc.sync.dma_start(out=W1[t][:, 0:2, :, :], in_=wd["W1" + t][:, 0:2, :, :])
            nc.sync.dma_start(out=x_sb[:, 4 * 18:8 * 18], in_=x_d[:, 4 * 18:8 * 18])
            for t in ("a", "b"):
                nc.sync.dma_start(out=W1[t][:, 2:8, :, :], in_=wd["W1" + t][:, 2:8, :, :])
            for t in ("a", "b"):
                nc.sync.dma_start(out=W2[t][:, 0:2, :, :], in_=wd["W2" + t][:, 0:2, :, :])

            def emit_bulk_dmas():
                nc.sync.dma_start(out=x_sb[:, 8 * 18:16 * 18], in_=x_d[:, 8 * 18:16 * 18])
                nc.sync.dma_start(out=x_sb[:, 16 * 18:], in_=x_d[:, 16 * 18:])
                for t in ("a", "b"):
                    nc.sync.dma_start(out=W2[t][:, 2:5, :, :], in_=wd["W2" + t][:, 2:5, :, :])
                for t in ("a", "b"):
                    nc.sync.dma_start(out=W2[t][:, 5:8, :, :], in_=wd["W2" + t][:, 5:8, :, :])
                    nc.sync.dma_start(out=W3[t][:], in_=wd["W3" + t][:, :])
                    nc.sync.dma_start(out=B2[t][:], in_=wd["b2" + t][:, :])
                    nc.sync.dma_start(out=B3[t], in_=wd["b3" + t].rearrange("(p o) -> p o", o=1))
                    nc.sync.dma_start(out=WF[t], in_=wd["Wf" + t][:, :])

            def feats_pair(g0):
                """Features for subtiles g0, g0+1 fused into single DVE ops
                wherever the access pattern fits the 3-free-dim limit (a
                leading [stride, 2] pair dim on every op that had <=2 free
                dims). Returns sec2 [128, 2, 768] bf16 (tail cols zeroed
                once per rotating buffer). The feats ops are tiny, so cost
                is instruction-count-dominated: pairing nearly halves it."""
                x2 = x_sb[:, g0 * 18: g0 * 18 + 36]
                # full 3x3 grids per factor: [128, s, 27], idx f*9+a*3+b
                XX2 = ft.tile([128, 2, 27], F32, tag="XX2", name="XX2")
                XXYY2 = ft.tile([128, 2, 27], F32, tag="XXYY2", name="XXYY2")
                XY2 = ft.tile([128, 2, 27], F32, tag="XY2", name="XY2")
                for s in range(2):
                    xr = x2[:, s * 18: s * 18 + 9]
                    xi = x2[:, s * 18 + 9: s * 18 + 18]
                    nc.vector.tensor_mul(XX2[:, s, :],
                                         _brd(xr, [[3, 3], [1, 3], [0, 3]], bass),
                                         _brd(xr, [[3, 3], [0, 3], [1, 3]], bass))
                    nc.vector.tensor_mul(XXYY2[:, s, :],
                                         _brd(xi, [[3, 3], [1, 3], [0, 3]], bass),
                                         _brd(xi, [[3, 3], [0, 3], [1, 3]], bass))
                    nc.vector.tensor_mul(XY2[:, s, :],
                                         _brd(xr, [[3, 3], [1, 3], [0, 3]], bass),
                                         _brd(xi, [[3, 3], [0, 3], [1, 3]], bass))
                nc.vector.tensor_add(XXYY2[:], XXYY2[:], XX2[:])

                # r_all2 [128, s, 3, 6]: triu cols {0,1,2},{4,5},{8} per grid
                r_all2 = ft.tile([128, 2, 3, 6], F32, tag="r_all2", name="r_all2")
                nc.vector.tensor_copy(_brd(r_all2[:, 0, 0, 0:3], [[18, 2], [6, 3], [1, 3]], bass),
                                      _brd(XXYY2[:, 0, 0:3], [[27, 2], [9, 3], [1, 3]], bass))
                nc.vector.tensor_copy(_brd(r_all2[:, 0, 0, 3:5], [[18, 2], [6, 3], [1, 2]], bass),
                                      _brd(XXYY2[:, 0, 4:6], [[27, 2], [9, 3], [1, 2]], bass))
                nc.vector.tensor_copy(_brd(r_all2[:, 0, 0, 5:6], [[18, 2], [6, 3], [1, 1]], bass),
                                      _brd(XXYY2[:, 0, 8:9], [[27, 2], [9, 3], [1, 1]], bass))
                # im_all2 [128, s, 3, 3]: XY[a,b]-XY[b,a] for (0,1),(0,2),(1,2)
                im_all2 = ft.tile([128, 2, 3, 3], F32, tag="im_all2", name="im_all2")
                nc.vector.tensor_sub(_brd(im_all2[:, 0, 0, 0:2], [[9, 2], [3, 3], [1, 2]], bass),
                                     _brd(XY2[:, 0, 1:3], [[27, 2], [9, 3], [1, 2]], bass),
                                     _brd(XY2[:, 0, 3:7], [[27, 2], [9, 3], [3, 2]], bass))
                nc.vector.tensor_sub(_brd(im_all2[:, 0, 0, 2:3], [[9, 2], [3, 3], [1, 1]], bass),
                                     _brd(XY2[:, 0, 5:6], [[27, 2], [9, 3], [1, 1]], bass),
                                     _brd(XY2[:, 0, 7:8], [[27, 2], [9, 3], [1, 1]], bass))
                # kappa [128, s, 3] = diag sums; kprod, inv
                kap2 = ft.tile([128, 2, 3], F32, tag="kap2", name="kap2")
                nc.vector.tensor_add(_brd(kap2[:, 0, 0:3], [[3, 2], [1, 3]], bass),
                                     _brd(XXYY2[:, 0, 0:1], [[27, 2], [9, 3]], bass),
                                     _brd(XXYY2[:, 0, 4:5], [[27, 2], [9, 3]], bass))
                nc.vector.tensor_add(_brd(kap2[:, 0, 0:3], [[3, 2], [1, 3]], bass),
                                     _brd(kap2[:, 0, 0:3], [[3, 2], [1, 3]], bass),
                                     _brd(XXYY2[:, 0, 8:9], [[27, 2], [9, 3]], bass))
                kp2 = ft.tile([128, 2], F32, tag="kp2", name="kp2")
                nc.vector.tensor_mul(kp2[:], _brd(kap2[:, 0, 0:1], [[3, 2]], bass),
                                     _brd(kap2[:, 0, 1:2], [[3, 2]], bass))
                # fold the fp8 feats scale here: inv = FP8_SF / kprod
                nc.vector.scalar_tensor_tensor(out=kp2[:], in0=kp2[:],
                                               scalar=1.0 / FP8_SF,
                                               in1=_brd(kap2[:, 0, 2:3], [[3, 2]], bass),
                                               op0=ALU.mult, op1=ALU.mult)
                inv2 = ft.tile([128, 2], F32, tag="inv2", name="inv2")
                nc.vector.reciprocal(inv2[:], kp2[:])

                # LF2 [128, s, 90]: level-2 vector L2 (81 = [R2 45 | I2n 36])
                # and kappa-folded factor-2 vector f2 (9 = [rr2 6 | ii2 3]).
                # i1 is used UN-negated; the -1 on L2-vec indices [36, 63)
                # is folded into the host-side W1 row signs.
                LF2 = ft.tile([128, 2, 90], F32, tag="LF2", name="LF2")
                r0 = r_all2[:, 0, 0, 0:6]
                r1 = r_all2[:, 0, 1, 0:6]
                r2 = r_all2[:, 0, 2, 0:6]
                i0 = im_all2[:, 0, 0, 0:3]
                i1 = im_all2[:, 0, 1, 0:3]
                i2 = im_all2[:, 0, 2, 0:3]
                nc.vector.tensor_mul(_brd(LF2[:, 0, 0:36], [[90, 2], [1, 36]], bass),
                                     _brd(r0, [[18, 2], [1, 6], [0, 6]], bass),
                                     _brd(r1, [[18, 2], [0, 6], [1, 6]], bass))
                nc.vector.tensor_mul(_brd(LF2[:, 0, 36:45], [[90, 2], [1, 9]], bass),
                                     _brd(i0, [[9, 2], [1, 3], [0, 3]], bass),
                                     _brd(i1, [[9, 2], [0, 3], [1, 3]], bass))
                nc.vector.tensor_mul(_brd(LF2[:, 0, 45:63], [[90, 2], [1, 18]], bass),
                                     _brd(r0, [[18, 2], [1, 6], [0, 3]], bass),
                                     _brd(i1, [[9, 2], [0, 6], [1, 3]], bass))
                nc.vector.tensor_mul(_brd(LF2[:, 0, 63:81], [[90, 2], [1, 18]], bass),
                                     _brd(i0, [[9, 2], [1, 3], [0, 6]], bass),
                                     _brd(r1, [[18, 2], [0, 3], [1, 6]], bass))
                nc.vector.tensor_mul(_brd(LF2[:, 0, 81:87], [[90, 2], [1, 6]], bass),
                                     _brd(r2, [[18, 2], [1, 6]], bass),
                                     _brd(inv2[:, 0:1], [[1, 2], [0, 6]], bass))
                nc.vector.tensor_mul(_brd(LF2[:, 0, 87:90], [[90, 2], [1, 3]], bass),
                                     _brd(i2, [[9, 2], [1, 3]], bass),
                                     _brd(inv2[:, 0:1], [[1, 2], [0, 3]], bass))

                sec2 = secp.tile([128, 2, 768], BF16, tag="sec", name="sec")
                # tail cols: col 729 is the constant 128 that multiplies the
                # b1 row folded into W1's padding (128*32*b1 = 4096*b1, the
                # exact ACT bias scale); 730.. stay zero. The pool rotates
                # through 4 buffers -> write each buffer once (first macros)
                if sec_zeroed[0] < 4:
                    nc.gpsimd.memset(sec2[:, :, 730:768], 0.0)
                    nc.gpsimd.memset(sec2[:, :, 729:730], 128.0)
                    sec_zeroed[0] += 1
                # sec[s, j*81+i] = L2[s, i] * f2[s, j] (f2 carries 128/kprod)
                nc.vector.tensor_mul(
                    _brd(sec2[:, 0, 0:729], [[768, 2], [81, 9], [1, 81]], bass),
                    _brd(LF2[:, 0, 0:81], [[90, 2], [0, 9], [1, 81]], bass),
                    _brd(LF2[:, 0, 81:90], [[90, 2], [1, 9], [0, 81]], bass))
                return sec2

            sec_zeroed = [0]

            def feats_T(mt):
                """DVE features -> sec [128, 768] bf16 (batch on partitions,
                cols 729.. zeroed) -> per-subtile SBUF->SBUF DMA transpose
                (the 16 DMA engines are otherwise idle; needs a 2-byte
                dtype, hence bf16) -> one DVE cast per subtile packs the
                transposed [128, 6, 128] staging tile into the fp8 featsT
                [128, 6, n_tile]. The chunk-5 tail rows (729..767) are
                zeros, so the DoubleRow pair (4,5) contracts cleanly
                against the 0-padded W1 tail. All secs+DMAs are emitted
                first, then the casts, so the in-order DVE queue never
                head-of-line blocks on DMA latency."""
                featsT = ftp.tile([128, 6, n_tile], FP8, tag="featsT", name="featsT")
                fts = []
                for sp in range(n_sub // 2):
                    sec2 = feats_pair(mt * n_sub + 2 * sp)
                    for s in range(2):
                        fT1 = ftsp.tile([128, 6, 128], BF16, tag="fT1", name="fT1")
                        nc.sync.dma_start_transpose(fT1[:], sec2[:, s, :])
                        fts.append(fT1)
                for s in range(n_sub):
                    nc.vector.tensor_copy(featsT[:, :, s * 128:(s + 1) * 128],
                                          fts[s][:])
                return featsT

            q1_scale = float(np.sqrt(FP8_SQ1) / (FP8_SF * FP8_SW))
            q2_scale = float(1.0 / (FP8_SQ1 * FP8_SW))

            def layer1(featsT, t):
                """L1 for one tower (only W1 weights needed). fp8 DoubleRow:
                each matmul contracts a pair of 128-row chunks. b1 rides in
                W1's pad row (sec col 729 = 128), so the ACT square is
                bias-free and drains TWO m-chunks (2 PSUM banks) per
                instruction — ACT cost is overhead-dominated at this size."""
                q1 = qp.tile([128, 8, n_tile], FP8, tag="q1" + t, name="q1" + t,
                             bufs=(2 if t == "a" else 1))
                for mp in range(4):
                    ps2 = psL.tile([128, 2, n_tile], F32, tag="mm", name="mm")
                    for j in range(2):
                        for k in range(3):
                            nc.tensor.matmul(ps2[:, j, :],
                                             W1[t][:, 2 * mp + j, 2 * k:2 * k + 2, :],
                                             featsT[:, 2 * k:2 * k + 2, :],
                                             start=(k == 0), stop=(k == 2),
                                             perf_mode=DR)
                    nc.scalar.activation(out=q1[:, 2 * mp:2 * mp + 2, :], in_=ps2[:],
                                         func=AF.Square, scale=q1_scale)
                return q1

            def tower_L2(t, q1):
                """L2 m-loop for one tower: 4 DR matmuls -> ACT square ->
                DVE W3-weighted accumulate into v[p, b] (one fused
                scalar_tensor_tensor per m-chunk; the GpSimd/Pool engine
                runs these ops in ~7.5us microcode, ~18x too slow). All
                bf16: 2x DVE throughput, and the partition-sum matmul is
                a 1-pass bf16 matmul (fp32 would be 2-pass LOW_HIGH).
                Returns v."""
                v = vp.tile([128, n_tile], BF16, tag="v" + t, name="v" + t)
                for m in range(8):
                    ps = psL.tile([128, n_tile], F32, tag="mm", name="mm")
                    for k in range(4):
                        nc.tensor.matmul(ps[:],
                                         W2[t][:, m, 2 * k:2 * k + 2, :],
                                         q1[:, 2 * k:2 * k + 2, :],
                                         start=(k == 0), stop=(k == 3),
                                         perf_mode=DR)
                    q2m = q2p.tile([128, n_tile], BF16, tag="q2m", name="q2m")
                    nc.scalar.activation(out=q2m[:], in_=ps[:], func=AF.Square,
                                         bias=B2[t][:, m:m + 1], scale=q2_scale)
                    if m == 0:
                        nc.vector.tensor_scalar_mul(v[:], q2m[:], W3[t][:, 0:1])
                    else:
                        nc.vector.scalar_tensor_tensor(
                            out=v[:], in0=q2m[:], scalar=W3[t][:, m:m + 1],
                            in1=v[:], op0=ALU.mult, op1=ALU.add)
                return v

            def tower_fin(t, v):
                """u3 = ones^T v (one PE matmul), then square+ln on ACT.
                Emit AFTER more PE work has been queued so the in-order PE
                doesn't head-of-line block on the ACT/GpSimd chain."""
                u3 = psU.tile([1, n_tile], F32, tag="u3", name="u3")
                nc.tensor.matmul(u3[:], ones[:], v[:], start=True, stop=True)
                if t == "a":
                    ln_t = ep.tile([1, n_tile], F32, tag="lna", name="lna")
                else:
                    # reuse a q2m slot (dead after the u3 matmul)
                    ln_t = q2p.tile([1, n_tile], F32, tag="q2m", name="lnb")
                nc.scalar.activation(out=ln_t[:], in_=u3[:], func=AF.Square,
                                     bias=B3[t], scale=1.0)
                nc.scalar.activation(out=ln_t[:], in_=ln_t[:], func=AF.Ln)
                return ln_t

            def layer23(mt, q1s):
                """L2 + W3 reduction for both towers; returns a finish
                closure (tower-b u3 matmul + combine + store) the caller
                emits after queueing the next macro's L1 matmuls."""
                base = mt * n_tile
                va = tower_L2("a", q1s["a"])
                vb = tower_L2("b", q1s["b"])
                # tower-a finish rides behind tower-b's L2 matmul stream
                lna = tower_fin("a", va)

                def finish():
                    lnb = tower_fin("b", vb)
                    nc.vector.tensor_scalar_mul(lnb[:], lnb[:], WF["b"])
                    nc.vector.scalar_tensor_tensor(out=lna[:], in0=lna[:],
                                                   scalar=WF["a"], in1=lnb[:],
                                                   op0=ALU.mult, op1=ALU.subtract)
                    nc.vector.tensor_scalar(out=lna[:], in0=lna[:], scalar1=1.0e6,
                                            scalar2=-1.0e6, op0=ALU.min, op1=ALU.max)
                    nc.sync.dma_start(out=out_d[base:base + n_tile], in_=lna[:])
                return finish

            # Software-pipelined emission. Features run one macro ahead
            # (featsT is double-buffered) and are emitted BETWEEN L1(mt) and
            # L23(mt): the PE stream is [L1 mms][transposes mt+1][L2 mms],
            # so by the time the PE reaches the transposes the DVE (which
            # computed the next macro's secs during L1) is done, and neither
            # engine head-of-line blocks on the other.
            fT = {0: feats_T(0)}
            emit_bulk_dmas()
            q1a_pre = {}
            fin_pend = None
            for mt in range(n_macro):
                cur = fT.pop(mt)
                q1a = q1a_pre.pop(mt) if mt in q1a_pre else layer1(cur, "a")
                q1b = layer1(cur, "b")
                if fin_pend is not None:
                    fin_pend()      # prev macro's tail, behind L1's matmuls
                if mt + 1 < n_macro:
                    fT[mt + 1] = feats_T(mt + 1)
                    if mt == 0:
                        # startup-only: macro 1's tower-a L1 needs just W1a,
                        # giving the PE work while the W2 DMAs land
                        q1a_pre[1] = layer1(fT[1], "a")
                fin_pend = layer23(mt, {"a": q1a, "b": q1b})
            fin_pend()

    if finalize:
        nc.finalize()   # Bacc pass pipeline: reg alloc, wait splitting, etc.
    return nc


def _w1_perm():
    """featsT row g = j*81 + i holds L2vec[i]*f2vec[j]; reference sec index
    for that product (blocks: R2xrr2 | I2nxii2 | R2xii2 | I2nxrr2)."""
    perm = np.empty(NSEC, np.int64)
    for j in range(9):
        for i in range(81):
            if i < 45:
                ref = i * 6 + j if j < 6 else 378 + i * 3 + (j - 6)
            else:
                ii = i - 45
                ref = 513 + ii * 6 + j if j < 6 else 270 + ii * 3 + (j - 6)
            perm[j * 81 + i] = ref
    return perm


def _to_fp8(w, scale):
    import ml_dtypes
    v = np.clip(np.asarray(w, np.float32) * np.float32(scale), -240.0, 240.0)
    return np.ascontiguousarray(v.astype(ml_dtypes.float8_e4m3))


def prep_weights(inputs, mm_dtype=MM_DTYPE):
    """Per-core weight dict, pre-tiled host-side to the kernel's exact SBUF
    layouts so every weight DMA is contiguous:
      W1 [128, 8m, 6k, 128c] fp8  (rows permuted to featsT order g=j*81+i,
          sign-folded for the dropped i1 negation, zero-padded to 768, *32)
      W2 [128, 8m, 8k, 128c] fp8  (*32)
      b1/b2/W3 [128, 8m] f32      (b1 scaled by sqrt(512), the q1 fp8 scale)
    """
    weights = {k: np.ascontiguousarray(np.asarray(v, dtype=np.float32))
               for k, v in inputs.items() if k != "x"}
    perm = _w1_perm()
    # the kernel computes LF with +i1 (no negation op); L2-vec indices
    # i in [36, 63) need a -1 that is folded into the W1 row signs here
    neg = (np.arange(NSEC) % 81 >= 36) & (np.arange(NSEC) % 81 < 63)
    for t in ("a", "b"):
        w1 = np.zeros((768, H), np.float32)
        w1[:NSEC] = weights["W1" + t][perm]
        w1[:NSEC][neg] *= -1.0
        # b1 rides in pad row 729: sec col 729 is the constant 128, so
        # 128 * (32*b1) = 4096*b1 = b1 / q1_scale * sqrt(512) exactly
        w1[NSEC] = weights.pop("b1" + t)
        w1 = _to_fp8(w1, FP8_SW)
        # [k*128+p, m*128+c] -> [p, m, k, c]
        weights["W1" + t] = np.ascontiguousarray(
            w1.reshape(6, 128, 8, 128).transpose(1, 2, 0, 3))
        w2 = _to_fp8(weights["W2" + t], FP8_SW)
        weights["W2" + t] = np.ascontiguousarray(
            w2.reshape(8, 128, 8, 128).transpose(1, 2, 0, 3))
        weights["b2" + t] = np.ascontiguousarray(
            weights["b2" + t].reshape(8, 128).T)
        weights["W3" + t] = np.ascontiguousarray(
            weights["W3" + t].reshape(8, 128).T)
    return weights


def prep_x(x_core):
    """Per-core x [b, 18] -> SBUF image [128, (b/128)*18]: partition p holds
    sample p of each 128-row subtile, subtiles concatenated along free dim."""
    b = x_core.shape[0]
    n_sub_total = b // 128
    return np.ascontiguousarray(
        x_core.reshape(n_sub_total, 128, 18).transpose(1, 0, 2).reshape(128, n_sub_total * 18))


def run(inputs, trace=False, b_core=B_CORE, n_tile=N_TILE, mm_dtype=MM_DTYPE,
        n_cores=N_CORES):
    """Shard inputs, run the SPMD kernel on n_cores, gather full output.
    Returns (out [B,1] fp32, BassKernelResults)."""
    from concourse import bass_utils

    x = np.ascontiguousarray(np.asarray(inputs["x"], dtype=np.float32))
    weights = prep_weights(inputs, mm_dtype)
    nc = build_nc(b_core=b_core, n_tile=n_tile, mm_dtype=mm_dtype)
    in_maps = []
    for c in range(n_cores):
        m = {"x": prep_x(x[c * b_core:(c + 1) * b_core])}
        m.update(weights)
        in_maps.append(m)
    res = bass_utils.run_bass_kernel_spmd(nc, in_maps, core_ids=list(range(n_cores)),
                                          trace=trace)
    out = np.concatenate([r["out"] for r in res.results], axis=0)
    return out.reshape(-1, 1).astype(np.float32), res


def kernel(**inputs) -> np.ndarray:
    out, _ = run(inputs, trace=False)
    return out

